# revision 12
# baseline (speedup 1.0000x reference)
"""Trainium2 Bass kernel for nn_CorrelationHead (8-core SPMD, data parallel over B).

Math (validated ~8e-7 fp32 / 3e-3 bf16 vs the jax reference):
  corr[b,p,q,i,j] = sum_c patch1[b,c,i,j] * patch2[b,c, i+2p-20, j+2q-20]
  out[b,n] = w[n,:]·corr[b,:] + bias[n]
           = sum_{ij,yx} G[b][yx,ij] * W3[n,ij,yx] + bias[n]
  where G[b] = patch2[b]^T patch1[b] (49x49 spatial Gram, contracted over
  the 128 channels) and W3 scatters w_bbox onto valid (ij,yx) pairs.

Per-core plan (64 samples), bf16, raw bass:
  - host packs ydat[128ch, 64 samples, 98] = [p1[b] | p2[b]] per sample,
    plus one weight tensor wdat[113, 400] (stacked stage-2 weights + bias
    column). Input DMA = 8 chunk-halves [128, 1568B contiguous rows]
    round-robined over the two HWDGE rings (sync+scalar); desc-gen is a
    flat ~750ns per dma_start so few big DMAs win.
  - PE warm-up: zero-matmuls clear PSUM rows 49-63 (so stage-2's dead
    partition rows read 0) and filler matmuls keep the PE busy so it
    p-state-ramps from 1.2 to 2.4 GHz before the real work.
  - stage 1: per sample pair one K=128 matmul per sample on alternating
    PE column strips (tile_position (0,0)/(0,64)) -> G[2t] at PSUM rows
    0-48, G[2t+1] at rows 64-112, 4 pairs per bank, all 32 pairs resident
    (no PSUM reuse, no cast backpressure).
  - cast: one DVE copy per chunk [113 parts, 2 banks, 196] -> acat bf16;
    113-partition copies halve the usual 49-partition cast cost.
  - stage 2: 49 accumulating matmuls per half, K=113 (rows 49-63 weight
    zero), M=8 (outputs 0-3 = even sample, 4-7 = odd sample of each
    pair), N=16 pairs.
  - tail: DVE tensor_scalar_add fuses the bias (per-partition scalar from
    the weight tensor's bias column) with the PSUM->SBUF copy; one 8-
    descriptor out DMA; no completion wait (the NEFF epilogue drains).
"""

import numpy as np

import concourse.bass as bass
import concourse.mybir as mybir
from concourse import bacc
from concourse.bass_utils import run_bass_kernel_spmd

N_CORES = 8
B, C, HW = 512, 128, 49
BS = B // N_CORES      # 64 samples per core
NP = BS // 2           # 32 pairs
NCH = 4                # data chunks
PAIRS_PER_CHUNK = NP // NCH   # 8
SAMP_PER_CHUNK = BS // NCH    # 16

_F32 = mybir.dt.float32
_BF16 = mybir.dt.bfloat16

N_ZERO_MM = 8
N_FILLER = 40


def build_nc() -> bass.Bass:
    nc = bacc.Bacc("TRN2", target_bir_lowering=False, debug=False)
    pp = nc.dram_tensor("pp", [C, BS, 2 * HW], _BF16, kind="ExternalInput")
    wd = nc.dram_tensor("wd", [113, 400], _BF16, kind="ExternalInput")
    out = nc.dram_tensor("out", [8, NP], _F32, kind="ExternalOutput")

    from contextlib import ExitStack

    with ExitStack() as ctx:
        ydat = ctx.enter_context(nc.sbuf_tensor("ydat", [C, BS, 2 * HW], _BF16))
        acat = ctx.enter_context(nc.sbuf_tensor("acat", [113, NP, HW], _BF16))
        wsb = ctx.enter_context(nc.sbuf_tensor("wsb", [113, 400], _BF16))
        zsb = ctx.enter_context(nc.sbuf_tensor("zsb", [1, 336], _BF16))
        out_sb = ctx.enter_context(nc.sbuf_tensor("out_sb", [8, NP], _F32))
        ps = ctx.enter_context(nc.psum_tensor("ps", [128, 8, 512], _F32))
        (sW, sZ, sMM, sC, sS2, sAdd, sDone) = (
            ctx.enter_context(nc.semaphore(nm))
            for nm in ("sW", "sZ", "sMM", "sC", "sS2", "sAdd", "sDone")
        )
        sDs = [
            ctx.enter_context(nc.semaphore(f"sDs{c}")) for c in range(NCH)
        ]
        sDa = [
            ctx.enter_context(nc.semaphore(f"sDa{c}")) for c in range(NCH)
        ]
        block = ctx.enter_context(nc.Block())

        # chunk c occupies samples 16c..16c+15 -> ydat cols, pp cols
        @block.sync
        def _(sync):
            sync.dma_start(out=wsb[:], in_=wd[:]).then_inc(sW, 16)
            for c in range(NCH):
                s0 = c * SAMP_PER_CHUNK
                sh = SAMP_PER_CHUNK // 2
                sync.dma_start(
                    out=ydat[:, s0 : s0 + sh, :], in_=pp[:, s0 : s0 + sh, :]
                ).then_inc(sDs[c], 16)
            sync.wait_ge(sAdd, 2)
            sync.dma_start(out=out[:], in_=out_sb[:]).then_inc(sDone, 16)
            sync.wait_ge(sDone, 16)

        @block.scalar
        def _(scalar):
            for c in range(NCH):
                s0 = c * SAMP_PER_CHUNK + SAMP_PER_CHUNK // 2
                sh = SAMP_PER_CHUNK // 2
                scalar.dma_start(
                    out=ydat[:, s0 : s0 + sh, :], in_=pp[:, s0 : s0 + sh, :]
                ).then_inc(sDa[c], 16)

        @block.tensor
        def _(tensor):
            tensor.wait_ge(sZ, 1)
            # zero PSUM cols 0:196 of every bank (rows 49-63 must read 0.0
            # in the casts; stage-1 start=True only resets rows it writes)
            for bank in range(8):
                nc.tensor.matmul(
                    ps[0:128, bank, 0:196],
                    zsb[0:1, 0:128],
                    zsb[0:1, 140:336],
                    start=True,
                    stop=True,
                    tile_position=(0, 0),
                )
            # p-state warm-up filler (dead compute into unused PSUM cols)
            for f in range(N_FILLER):
                nc.tensor.matmul(
                    ps[0:128, 2 + (f % 6), 448:512],
                    zsb[0:1, 0:128],
                    zsb[0:1, 140:204],
                    start=True,
                    stop=True,
                    tile_position=(0, 0),
                )

            def stage1(c):
                tensor.wait_ge(sDs[c], 16)
                tensor.wait_ge(sDa[c], 16)
                for k in range(PAIRS_PER_CHUNK):
                    t = c * PAIRS_PER_CHUNK + k
                    bank, off = t // 4, (t % 4) * HW
                    bA, bB = 2 * t, 2 * t + 1
                    nc.tensor.matmul(
                        ps[0:49, bank, off : off + HW],
                        ydat[:, bA, HW : 2 * HW],
                        ydat[:, bA, 0:HW],
                        start=True,
                        stop=True,
                        tile_position=(0, 0),
                    )
                    mm = nc.tensor.matmul(
                        ps[64:113, bank, off : off + HW],
                        ydat[:, bB, HW : 2 * HW],
                        ydat[:, bB, 0:HW],
                        start=True,
                        stop=True,
                        tile_position=(0, 64),
                    )
                    if k == PAIRS_PER_CHUNK - 1:
                        mm.then_inc(sMM, 1)

            def stage2(h):
                # half h: pairs 16h..16h+15, accumulate into ps[0:8, h, 448:464]
                for ij in range(HW):
                    mm = nc.tensor.matmul(
                        ps[0:8, h, 448 : 448 + 16],
                        wsb[:, ij * 8 : ij * 8 + 8],
                        acat[:, 16 * h : 16 * h + 16, ij],
                        start=(ij == 0),
                        stop=(ij == HW - 1),
                        tile_position=(0, 0),
                    )
                mm.then_inc(sS2, 1)

            stage1(0)
            stage1(1)
            tensor.wait_ge(sW, 16)
            tensor.wait_ge(sC, 2)
            stage2(0)
            stage1(2)
            stage1(3)
            tensor.wait_ge(sC, 4)
            stage2(1)

        @block.vector
        def _(vector):
            nc.vector.memset(zsb[:], 0.0).then_inc(sZ, 1)
            for c in range(NCH):
                vector.wait_ge(sMM, c + 1)
                nc.vector.tensor_copy(
                    acat[:, c * PAIRS_PER_CHUNK : (c + 1) * PAIRS_PER_CHUNK, :],
                    ps[0:113, 2 * c : 2 * c + 2, 0:196],
                ).then_inc(sC, 1)
            for h in range(2):
                vector.wait_ge(sS2, h + 1)
                nc.vector.tensor_scalar_add(
                    out_sb[:, 16 * h : 16 * h + 16],
                    ps[0:8, h, 448 : 448 + 16],
                    wsb[0:8, 392:394].bitcast(_F32),
                ).then_inc(sAdd, 1)

    nc.compile()
    return nc


def _build_wd(w_bbox: np.ndarray, b_bbox: np.ndarray) -> np.ndarray:
    W3 = np.zeros((4, 49, 49), np.float32)
    for i in range(7):
        for j in range(7):
            for y in range(7):
                for x in range(7):
                    if (y - i) % 2 == 0 and (x - j) % 2 == 0:
                        p = (y - i + 20) // 2
                        q = (x - j + 20) // 2
                        W3[:, i * 7 + j, y * 7 + x] = w_bbox[
                            :, ((p * 21 + q) * 7 + i) * 7 + j
                        ]
    import ml_dtypes

    bf = ml_dtypes.bfloat16
    wd = np.zeros((113, 400), np.float32)
    # wd[yx, ij*8 + n]: rows 0-48 -> outputs 0-3; rows 64-112 -> outputs 4-7
    for ij in range(49):
        wd[0:49, ij * 8 : ij * 8 + 4] = W3[:, ij, :].T
        wd[64:113, ij * 8 + 4 : ij * 8 + 8] = W3[:, ij, :].T
    wdb = wd.astype(bf)
    # cols 392-393 carry the fp32 bias bit-pattern (kernel bitcasts to f32)
    wdb[0:8, 392:394] = (
        np.tile(b_bbox, 2).astype(np.float32).copy().view(np.uint16).view(bf).reshape(8, 2)
    )
    return wdb


def _prep_inputs(inputs):
    import ml_dtypes

    bf = ml_dtypes.bfloat16
    p1 = np.asarray(inputs["patch1"], np.float32).reshape(B, C, HW)
    p2 = np.asarray(inputs["patch2"], np.float32).reshape(B, C, HW)
    # ydat[c, ch, b_local, 0:49]=p1, [.., 49:98]=p2
    Y = np.empty((B, C, 2 * HW), np.float32)
    Y[:, :, 0:HW] = p1
    Y[:, :, HW:] = p2
    Yb = (
        Y.reshape(N_CORES, BS, C, 2 * HW).transpose(0, 2, 1, 3).astype(bf)
    )  # [core, C, BS, 98]
    wdf = _build_wd(
        np.asarray(inputs["w_bbox"], np.float32),
        np.asarray(inputs["b_bbox"], np.float32),
    )
    in_maps = []
    for c in range(N_CORES):
        in_maps.append({"pp": np.ascontiguousarray(Yb[c]), "wd": wdf})
    return in_maps


def _run(inputs, trace: bool = False):
    nc = build_nc()
    in_maps = _prep_inputs(inputs)
    res = run_bass_kernel_spmd(
        nc, in_maps, core_ids=list(range(N_CORES)), trace=trace
    )
    # res out [8, 32]: out[4h+n, t] = sample 2t+h, output n
    outs = []
    for c in range(N_CORES):
        r = res.results[c]["out"].astype(np.float32).reshape(2, 4, NP)
        outs.append(r.transpose(2, 0, 1).reshape(BS, 4))
    return np.concatenate(outs, axis=0), res


def kernel(**inputs) -> np.ndarray:
    out, _ = _run(inputs, trace=False)
    return out


# revision 14
# speedup vs baseline: 1.0516x; 1.0516x over previous
"""Trainium2 Bass kernel for nn_CorrelationHead (8-core SPMD, data parallel over B).

Math (validated ~8e-7 fp32 / 3e-3 bf16 vs the jax reference):
  corr[b,p,q,i,j] = sum_c patch1[b,c,i,j] * patch2[b,c, i+2p-20, j+2q-20]
  out[b,n] = w[n,:]·corr[b,:] + bias[n]
           = sum_{ij,yx} G[b][yx,ij] * W3[n,ij,yx] + bias[n]
  where G[b] = patch2[b]^T patch1[b] (49x49 spatial Gram, contracted over
  the 128 channels) and W3 scatters w_bbox onto valid (ij,yx) pairs.

Per-core plan (64 samples), bf16, raw bass:
  - host packs ydat[128ch, 64 samples, 98] = [p1[b] | p2[b]] per sample,
    plus one weight tensor wdat[113, 400] (stacked stage-2 weights + bias
    column). Input DMA = 8 chunk-halves [128, 1568B contiguous rows]
    round-robined over the two HWDGE rings (sync+scalar); desc-gen is a
    flat ~750ns per dma_start so few big DMAs win.
  - PE warm-up: zero-matmuls clear PSUM rows 49-63 (so stage-2's dead
    partition rows read 0) and filler matmuls keep the PE busy so it
    p-state-ramps from 1.2 to 2.4 GHz before the real work.
  - stage 1: per sample pair one K=128 matmul per sample on alternating
    PE column strips (tile_position (0,0)/(0,64)) -> G[2t] at PSUM rows
    0-48, G[2t+1] at rows 64-112, 4 pairs per bank, all 32 pairs resident
    (no PSUM reuse, no cast backpressure).
  - cast: one DVE copy per chunk [113 parts, 2 banks, 196] -> acat bf16;
    113-partition copies halve the usual 49-partition cast cost.
  - stage 2: 49 accumulating matmuls per half, K=113 (rows 49-63 weight
    zero), M=8 (outputs 0-3 = even sample, 4-7 = odd sample of each
    pair), N=16 pairs.
  - tail: DVE tensor_scalar_add fuses the bias (per-partition scalar from
    the weight tensor's bias column) with the PSUM->SBUF copy; one 8-
    descriptor out DMA; no completion wait (the NEFF epilogue drains).
"""

import numpy as np

import concourse.bass as bass
import concourse.mybir as mybir
from concourse import bacc
from concourse.bass_utils import run_bass_kernel_spmd

N_CORES = 8
B, C, HW = 512, 128, 49
BS = B // N_CORES      # 64 samples per core
NP = BS // 2           # 32 pairs
NCH = 4                # data chunks
PAIRS_PER_CHUNK = NP // NCH   # 8
SAMP_PER_CHUNK = BS // NCH    # 16

_F32 = mybir.dt.float32
_BF16 = mybir.dt.bfloat16

N_ZERO_MM = 8
N_FILLER = 40


def build_nc() -> bass.Bass:
    nc = bacc.Bacc("TRN2", target_bir_lowering=False, debug=False)
    pp = nc.dram_tensor("pp", [C, BS, 2 * HW], _BF16, kind="ExternalInput")
    wd = nc.dram_tensor("wd", [113, 400], _BF16, kind="ExternalInput")
    out = nc.dram_tensor("out", [8, NP], _F32, kind="ExternalOutput")

    from contextlib import ExitStack

    with ExitStack() as ctx:
        ydat = ctx.enter_context(nc.sbuf_tensor("ydat", [C, BS, 2 * HW], _BF16))
        acat = ctx.enter_context(nc.sbuf_tensor("acat", [113, NP, HW], _BF16))
        wsb = ctx.enter_context(nc.sbuf_tensor("wsb", [113, 400], _BF16))
        zsb = ctx.enter_context(nc.sbuf_tensor("zsb", [1, 336], _BF16))
        out_sb = ctx.enter_context(nc.sbuf_tensor("out_sb", [8, NP], _F32))
        tmp_sb = ctx.enter_context(nc.sbuf_tensor("tmp_sb", [8, NP], _F32))
        ps = ctx.enter_context(nc.psum_tensor("ps", [128, 8, 512], _F32))
        (sW, sZ, sMM, sC, sS2, sAdd, sDone) = (
            ctx.enter_context(nc.semaphore(nm))
            for nm in ("sW", "sZ", "sMM", "sC", "sS2", "sAdd", "sDone")
        )
        sDs = [
            ctx.enter_context(nc.semaphore(f"sDs{c}")) for c in range(2)
        ]
        sDa = [
            ctx.enter_context(nc.semaphore(f"sDa{c}")) for c in range(2)
        ]
        block = ctx.enter_context(nc.Block())

        # chunk c occupies samples 16c..16c+15 -> ydat cols, pp cols
        @block.sync
        def _(sync):
            sync.dma_start(out=wsb[:], in_=wd[:]).then_inc(sW, 16)
            for h in range(2):
                s0 = 32 * h
                sync.dma_start(
                    out=ydat[:, s0 : s0 + 16, :], in_=pp[:, s0 : s0 + 16, :]
                ).then_inc(sDs[h], 16)
            sync.wait_ge(sAdd, 2)
            sync.dma_start(out=out[:], in_=out_sb[:]).then_inc(sDone, 16)
            sync.wait_ge(sDone, 16)

        @block.scalar
        def _(scalar):
            for h in range(2):
                s0 = 32 * h + 16
                scalar.dma_start(
                    out=ydat[:, s0 : s0 + 16, :], in_=pp[:, s0 : s0 + 16, :]
                ).then_inc(sDa[h], 16)

        @block.tensor
        def _(tensor):
            tensor.wait_ge(sZ, 1)
            # zero PSUM cols 0:196 of every bank (rows 49-63 must read 0.0
            # in the casts; stage-1 start=True only resets rows it writes)
            for bank in range(8):
                nc.tensor.matmul(
                    ps[0:128, bank, 0:196],
                    zsb[0:1, 0:128],
                    zsb[0:1, 140:336],
                    start=True,
                    stop=True,
                    tile_position=(0, 0),
                )
            # (p-state filler removed: measured no ramp on HW)

            def stage1(h):
                # half h: pairs 16h..16h+15 (samples 32h..32h+31)
                tensor.wait_ge(sDs[h], 16)
                tensor.wait_ge(sDa[h], 16)
                for k in range(16):
                    t = 16 * h + k
                    bank, off = t // 4, (t % 4) * HW
                    bA, bB = 2 * t, 2 * t + 1
                    nc.tensor.matmul(
                        ps[0:49, bank, off : off + HW],
                        ydat[:, bA, HW : 2 * HW],
                        ydat[:, bA, 0:HW],
                        start=True,
                        stop=True,
                        tile_position=(0, 0),
                    )
                    mm = nc.tensor.matmul(
                        ps[64:113, bank, off : off + HW],
                        ydat[:, bB, HW : 2 * HW],
                        ydat[:, bB, 0:HW],
                        start=True,
                        stop=True,
                        tile_position=(0, 64),
                    )
                    if k % 8 == 7:
                        mm.then_inc(sMM, 1)

            def stage2(h):
                # two ij-parity chains on PE column strips (0,0)/(0,32) so
                # LDWEIGHTS of one chain hides under the other's stream
                bank = 4 * h
                order = []
                for u in range(25):
                    order.append(2 * u)
                    if 2 * u + 1 < HW:
                        order.append(2 * u + 1)
                for i, ij in enumerate(order):
                    par = ij % 2
                    mm = nc.tensor.matmul(
                        ps[32 * par : 32 * par + 8, bank, 448 : 448 + 16],
                        wsb[:, ij * 8 : ij * 8 + 8],
                        acat[:, 16 * h : 16 * h + 16, ij],
                        start=(i < 2),
                        stop=(i >= len(order) - 2),
                        tile_position=(0, 32 * par),
                    )
                    if i >= len(order) - 2:
                        mm.then_inc(sS2, 1)

            stage1(0)
            tensor.wait_ge(sW, 16)
            tensor.wait_ge(sC, 2)
            stage2(0)
            stage1(1)
            tensor.wait_ge(sC, 4)
            stage2(1)

        @block.vector
        def _(vector):
            nc.vector.memset(zsb[:], 0.0).then_inc(sZ, 1)
            for q in range(4):
                vector.wait_ge(sMM, q + 1)
                nc.vector.tensor_copy(
                    acat[:, q * 8 : (q + 1) * 8, :],
                    ps[0:113, 2 * q : 2 * q + 2, 0:196],
                ).then_inc(sC, 1)
            import concourse.mybir as _mb
            for h in range(2):
                vector.wait_ge(sS2, 2 * (h + 1))
                nc.vector.tensor_copy(
                    tmp_sb[:, 16 * h : 16 * h + 16],
                    ps[32:40, 4 * h, 448 : 448 + 16],
                )
                nc.vector.scalar_tensor_tensor(
                    out_sb[:, 16 * h : 16 * h + 16],
                    ps[0:8, 4 * h, 448 : 448 + 16],
                    wsb[0:8, 392:394].bitcast(_F32),
                    tmp_sb[:, 16 * h : 16 * h + 16],
                    _mb.AluOpType.add,
                    _mb.AluOpType.add,
                ).then_inc(sAdd, 1)

    nc.compile()
    return nc


def _build_wd(w_bbox: np.ndarray, b_bbox: np.ndarray) -> np.ndarray:
    W3 = np.zeros((4, 49, 49), np.float32)
    for i in range(7):
        for j in range(7):
            for y in range(7):
                for x in range(7):
                    if (y - i) % 2 == 0 and (x - j) % 2 == 0:
                        p = (y - i + 20) // 2
                        q = (x - j + 20) // 2
                        W3[:, i * 7 + j, y * 7 + x] = w_bbox[
                            :, ((p * 21 + q) * 7 + i) * 7 + j
                        ]
    import ml_dtypes

    bf = ml_dtypes.bfloat16
    wd = np.zeros((113, 400), np.float32)
    # wd[yx, ij*8 + n]: rows 0-48 -> outputs 0-3; rows 64-112 -> outputs 4-7
    for ij in range(49):
        wd[0:49, ij * 8 : ij * 8 + 4] = W3[:, ij, :].T
        wd[64:113, ij * 8 + 4 : ij * 8 + 8] = W3[:, ij, :].T
    wdb = wd.astype(bf)
    # cols 392-393 carry the fp32 bias bit-pattern (kernel bitcasts to f32)
    wdb[0:8, 392:394] = (
        np.tile(b_bbox, 2).astype(np.float32).copy().view(np.uint16).view(bf).reshape(8, 2)
    )
    return wdb


def _prep_inputs(inputs):
    import ml_dtypes

    bf = ml_dtypes.bfloat16
    p1 = np.asarray(inputs["patch1"], np.float32).reshape(B, C, HW)
    p2 = np.asarray(inputs["patch2"], np.float32).reshape(B, C, HW)
    # ydat[c, ch, b_local, 0:49]=p1, [.., 49:98]=p2
    Y = np.empty((B, C, 2 * HW), np.float32)
    Y[:, :, 0:HW] = p1
    Y[:, :, HW:] = p2
    Yb = (
        Y.reshape(N_CORES, BS, C, 2 * HW).transpose(0, 2, 1, 3).astype(bf)
    )  # [core, C, BS, 98]
    wdf = _build_wd(
        np.asarray(inputs["w_bbox"], np.float32),
        np.asarray(inputs["b_bbox"], np.float32),
    )
    in_maps = []
    for c in range(N_CORES):
        in_maps.append({"pp": np.ascontiguousarray(Yb[c]), "wd": wdf})
    return in_maps


def _run(inputs, trace: bool = False):
    nc = build_nc()
    in_maps = _prep_inputs(inputs)
    res = run_bass_kernel_spmd(
        nc, in_maps, core_ids=list(range(N_CORES)), trace=trace
    )
    # res out [8, 32]: out[4h+n, t] = sample 2t+h, output n
    outs = []
    for c in range(N_CORES):
        r = res.results[c]["out"].astype(np.float32).reshape(2, 4, NP)
        outs.append(r.transpose(2, 0, 1).reshape(BS, 4))
    return np.concatenate(outs, axis=0), res


def kernel(**inputs) -> np.ndarray:
    out, _ = _run(inputs, trace=False)
    return out
